# revision 6
# baseline (speedup 1.0000x reference)
"""Trainium2 Bass kernel for nn_CustomAttention (B=16, R=128, D=128, BD=64).

Sharding: Wq (R,R,D,D) is split along the target-region axis s across the
8 cores (16 s-values per core).  Each core computes its slice of
Q/scores/attended; kv_embed and the shared K/V projections are replicated.

Per-core device layout: the 16 local s-values are processed as 4 "quads";
a quad's 4 members occupy 16-row blocks of PSUM at partition bases
0/32/64/96 (PE column-group alignment -- only 32-aligned output bases are
legal), with batch b in the 16 rows of each block.  The gap rows carry
benign garbage that is never read downstream.

The Wq stream (the memory-bound term) is shipped as a SINGLE bf16 plane
(64 MiB/core -- half the fp32 bytes; fp32 PSUM accumulation keeps the
final error well inside the 2e-2 gate), laid out host-side as
[tcx][i][s][(t,d)] so each t-chunk is one contiguous 4 MiB block, fetched
as two 2 MiB DMAs on the two HWDGE rings (sync + scalar).

K is produced per t-chunk inside the main loop, directly in the score
layout (only partition rows 32j+b are ever read): per t, four col-tiled
matmuls against strided kv slices; bk is folded into the PSUM->SBUF
epilogue add on DVE.  Scores: ACT copies each PSUM Q-block to SBUF bf16,
DVE does the bf16 mul (2x mode) + one 1024-wide segmented reduce per
(tcx,q).  V path runs early to fill the stream lead-in.
"""

import numpy as np
import ml_dtypes

try:
    import concourse  # noqa: F401
except ImportError:  # pragma: no cover
    import sys

    sys.path.insert(0, "/opt/trn_rl_repo")

from contextlib import ExitStack

import concourse.mybir as mybir
import concourse.tile as tile
from concourse import bacc
from concourse.bass_utils import run_bass_kernel_spmd
from concourse.masks import make_identity

F32 = mybir.dt.float32
F32R = mybir.dt.float32r
BF16 = mybir.dt.bfloat16
AF = mybir.ActivationFunctionType
ALU = mybir.AluOpType
AXL = mybir.AxisListType
NPBF16 = ml_dtypes.bfloat16

B, R, D, BD = 16, 128, 128, 64
NCORES = 8
SLOC = R // NCORES          # 16 s-values per core
NQ = SLOC // 4              # 4 quads of 4 members
TCH = 8                     # t-values per DMA chunk
NTC = R // TCH              # 16 chunks
GP = TCH // 4               # psum groups (of 4 t) per chunk
CHF = TCH * D               # free elements per chunk per s (1024)
SH = SLOC // 2              # s-values per half-DMA

_CACHE = {}


def _build():
    nc = bacc.Bacc("TRN2", target_bir_lowering=False, debug=False,
                   enable_asserts=True, num_devices=NCORES)

    def dram_in(name, shape, dt):
        return nc.dram_tensor(name, shape, dt, kind="ExternalInput").ap()

    wqh_ap = dram_in("wqh", [NTC, D, SLOC * CHF], BF16)  # [tc][i][(s,t,d)]
    qt_ap = dram_in("qt", [D, SLOC * B], BF16)          # [i][(s,b)]
    bq_ap = dram_in("bqd", [SLOC, R * D], BF16)         # [s][(t,d)]
    kvtb_ap = dram_in("kvtb", [D, B * R], BF16)         # [i][(b,t)]
    wkb_ap = dram_in("wkb", [D, D], BF16)
    bk5_ap = dram_in("bk5", [128, 512], F32)            # bk tiled 4x along free
    wvb_ap = dram_in("wvb", [D, BD], BF16)
    wr_ap = dram_in("wr", [BD, D], F32R)
    sel_ap = dram_in("sel", [SLOC, SLOC * B], BF16)     # bias row selectors (K=16)
    bv_ap = dram_in("bvr", [R, BD], F32)
    br_ap = dram_in("brr", [R, D], F32)

    awout_ap = nc.dram_tensor("awout", [4, B, NQ * R], F32,
                              kind="ExternalOutput").ap()
    attout_ap = nc.dram_tensor("attout", [2, 128, D], F32,
                               kind="ExternalOutput").ap()

    with tile.TileContext(nc) as tc:
        with ExitStack() as ctx:
            per = ctx.enter_context(tc.tile_pool(name="persist", bufs=1))
            pre = ctx.enter_context(tc.tile_pool(name="prelude", bufs=2))
            wqp = ctx.enter_context(tc.tile_pool(name="wqpool", bufs=3))
            qsp = ctx.enter_context(tc.tile_pool(name="qspool", bufs=3))
            tmpp = ctx.enter_context(tc.tile_pool(name="tmppool", bufs=2))
            qpsp = ctx.enter_context(tc.tile_pool(name="qps", bufs=4, space="PSUM"))
            aux = ctx.enter_context(tc.tile_pool(name="aux", bufs=2, space="PSUM"))
            kaux = ctx.enter_context(tc.tile_pool(name="kaux", bufs=1, space="PSUM"))
            aux1 = ctx.enter_context(tc.tile_pool(name="aux1", bufs=1, space="PSUM"))

            # ---- small inputs (sync ring, ahead of the wq stream) ----
            bq_all = per.tile([SLOC, R * D], BF16)
            qt = per.tile([D, SLOC * B], BF16)
            sel = per.tile([SLOC, SLOC * B], BF16)
            wkb = per.tile([D, D], BF16)
            bk5 = per.tile([128, 512], F32)
            kvtb = per.tile([D, B * R], BF16)
            wvb = per.tile([D, BD], BF16)
            wr = per.tile([BD, D], F32R)
            bv_rep = per.tile([R, BD], F32)
            br_rep = per.tile([R, D], F32)
            ident = per.tile([128, 128], F32)
            for t, ap in ((bq_all, bq_ap), (qt, qt_ap), (sel, sel_ap),
                          (wkb, wkb_ap), (bk5, bk5_ap), (kvtb, kvtb_ap),
                          (wvb, wvb_ap), (wr, wr_ap), (bv_rep, bv_ap),
                          (br_rep, br_ap)):
                nc.sync.dma_start(t[:], ap[:])
            make_identity(nc, ident[:])

            kvt3 = kvtb[:].rearrange("i (b t) -> i t b", b=B)  # strided view

            # ---- V path early: V = kv@Wv + bv, row-normalized ----
            vn = per.tile([R, B * BD], F32R)            # V_norm[b] as [t, (b,dd)]
            for b in range(B):
                vb_ps = aux.tile([128, BD], F32, tag="auxA")
                nc.tensor.matmul(vb_ps[:], kvtb[:, b * R:(b + 1) * R], wvb[:],
                                 start=True, stop=True)
                vsb = pre.tile([R, BD], F32, tag="vsb")
                nc.vector.tensor_add(vsb[:], vb_ps[:], bv_rep[:])
                vsq = pre.tile([R, BD], F32, tag="vsq")
                ss = pre.tile([R, 1], F32, tag="ss")
                nc.scalar.activation(vsq[:], vsb[:], AF.Square, accum_out=ss[:])
                nrm = pre.tile([R, 1], F32, tag="nrm")
                nc.scalar.activation(nrm[:], ss[:], AF.Sqrt)
                nc.vector.tensor_scalar_max(nrm[:], nrm[:], 1e-12)
                vri = pre.tile([R, 1], F32, tag="vri")
                nc.vector.reciprocal(vri[:], nrm[:])
                nc.vector.tensor_scalar_mul(vn[:, b * BD:(b + 1) * BD], vsb[:], vri[:])

            aw_raw = per.tile([128, NQ * R], F32)       # row 32j+b, col q*128+t

            # ---- main loop: per t-chunk build K slice, stream Wq, scores ----
            kreps = [per.tile([128, CHF], BF16, name=f"krep{i}",
                              tag=f"krep{i}") for i in range(NTC)]
            for tcx in range(NTC):
                # K[b,t,d] into partition rows 32j+b (other rows: garbage)
                krep = kreps[tcx]
                for g in range(GP):
                    kps = kaux.tile([128, 512], F32, tag="kps")
                    for v in range(4):
                        t = tcx * TCH + g * 4 + v
                        for jj in range(4):
                            nc.tensor.matmul(
                                kps[32 * jj:32 * jj + B, 128 * v:128 * (v + 1)],
                                kvt3[:, t, :], wkb[:],
                                start=True, stop=True,
                                tile_position=(0, 32 * jj))
                    nc.vector.tensor_add(krep[:, g * 512:(g + 1) * 512],
                                         kps[:], bk5[:])

                wq = wqp.tile([D, SLOC * CHF], BF16, tag="wqt")
                half = SH * CHF
                nc.sync.dma_start(wq[:, 0:half], wqh_ap[tcx, :, 0:half])
                nc.scalar.dma_start(wq[:, half:2 * half],
                                    wqh_ap[tcx, :, half:2 * half])
                for q in range(NQ):
                    tmp = tmpp.tile([128, CHF], BF16, tag="tmp")
                    for g in range(GP):
                        qps = qpsp.tile([128, 512], F32, tag="qps")
                        for j in range(4):
                            s = 4 * q + j
                            out = qps[32 * j:32 * j + B, :]
                            lhsq = qt[:, s * B:(s + 1) * B]
                            csl = slice(s * CHF + g * 512, s * CHF + (g + 1) * 512)
                            nc.tensor.matmul(out, lhsq, wq[:, csl],
                                             start=True, stop=False,
                                             tile_position=(0, 32 * j))
                            nc.tensor.matmul(out, sel[:, s * B:(s + 1) * B],
                                             bq_all[:, tcx * CHF + g * 512:
                                                    tcx * CHF + (g + 1) * 512],
                                             start=False, stop=True,
                                             tile_position=(0, 32 * j))
                        qs = qsp.tile([128, 512], BF16, tag="qs")
                        nc.scalar.activation(qs[:], qps[:], AF.Copy)
                        nc.vector.tensor_mul(tmp[:, g * 512:(g + 1) * 512],
                                             qs[:], krep[:, g * 512:(g + 1) * 512])
                    col = q * R + tcx * TCH
                    nc.vector.tensor_reduce(
                        aw_raw[:, col:col + TCH],
                        tmp[:].rearrange("p (t d) -> p t d", t=TCH),
                        axis=AXL.X, op=ALU.add)

            # ---- postlude: relu, normalize, attended, output proj ----
            aw_norm = per.tile([128, NQ * R], F32)
            ssum = per.tile([128, NQ], F32)
            rden = per.tile([128, NQ], F32)
            rinv = per.tile([128, NQ], F32)
            awt = per.tile([128, NQ * R], F32R)

            for q in range(NQ):
                nc.scalar.activation(aw_norm[:, q * R:(q + 1) * R],
                                     aw_raw[:, q * R:(q + 1) * R], AF.Relu,
                                     accum_out=ssum[:, q:q + 1])
            nc.vector.tensor_scalar_add(rden[:], ssum[:], 1e-8)
            nc.vector.reciprocal(rinv[:], rden[:])
            for q in range(NQ):
                nc.scalar.activation(aw_norm[:, q * R:(q + 1) * R],
                                     aw_norm[:, q * R:(q + 1) * R], AF.Copy,
                                     scale=rinv[:, q:q + 1])
            for j in range(4):
                nc.sync.dma_start(awout_ap[j], aw_norm[32 * j:32 * j + B, :])
            for q in range(NQ):
                trp = aux.tile([128, 128], F32, tag="auxA")
                nc.tensor.transpose(trp[:], aw_norm[:, q * R:(q + 1) * R], ident[:])
                nc.vector.tensor_copy(awt[:, q * R:(q + 1) * R], trp[:])

            attps = aux1.tile([BD, B * SLOC], F32, tag="auxC")
            awt4 = awt[:].rearrange("p (q j c) -> p q j c", q=NQ, j=4, c=32)
            for b in range(B):
                nc.tensor.matmul(attps[:, b * SLOC:(b + 1) * SLOC],
                                 vn[:, b * BD:(b + 1) * BD], awt4[:, :, :, b],
                                 start=True, stop=True)
            atts = per.tile([BD, B * SLOC], F32R)
            nc.vector.tensor_copy(atts[:], attps[:])

            for h in range(2):
                finps = aux.tile([128, D], F32, tag="auxA")
                nc.tensor.matmul(finps[:], atts[:, h * 128:(h + 1) * 128], wr[:],
                                 start=True, stop=True)
                fin = per.tile([128, D], F32, tag=f"fin{h}")
                nc.vector.tensor_add(fin[:], finps[:], br_rep[:])
                nc.sync.dma_start(attout_ap[h], fin[:])

    nc.compile()
    return nc


def _get_nc():
    if "nc" not in _CACHE:
        _CACHE["nc"] = _build()
    return _CACHE["nc"]


def _make_in_maps(q_embed, kv_embed, Wk, bk, Wv, bv, Wq, bq, Wr, br):
    sel = np.zeros((SLOC, SLOC * B), NPBF16)
    for s in range(SLOC):
        sel[s, s * B:(s + 1) * B] = 1.0
    shared = {
        "kvtb": np.ascontiguousarray(
            kv_embed.transpose(2, 0, 1).reshape(D, B * R)).astype(NPBF16),
        "wkb": np.ascontiguousarray(Wk).astype(NPBF16),
        "bk5": np.ascontiguousarray(
            np.broadcast_to(np.tile(bk, 4), (128, 512))).astype(np.float32),
        "wvb": np.ascontiguousarray(Wv).astype(NPBF16),
        "wr": np.ascontiguousarray(Wr),
        "sel": sel,
        "bvr": np.ascontiguousarray(np.broadcast_to(bv, (R, BD))),
        "brr": np.ascontiguousarray(np.broadcast_to(br, (R, D))),
    }
    in_maps = []
    for c in range(NCORES):
        s0 = SLOC * c
        m = dict(shared)
        # [s,t,i,d] -> [tcx, i, s, tch, d] so each tcx slice is one
        # contiguous 4 MiB bf16 block with i on partitions
        wq_c = (Wq[s0:s0 + SLOC]
                .reshape(SLOC, NTC, TCH, D, D)
                .transpose(1, 3, 0, 2, 4)
                .reshape(NTC, D, SLOC * CHF))
        m["wqh"] = np.ascontiguousarray(wq_c).astype(NPBF16)
        m["qt"] = np.ascontiguousarray(
            q_embed[:, s0:s0 + SLOC, :].transpose(2, 1, 0)).reshape(
                D, SLOC * B).astype(NPBF16)
        m["bqd"] = np.ascontiguousarray(
            bq[s0:s0 + SLOC]).reshape(SLOC, R * D).astype(NPBF16)
        in_maps.append(m)
    return in_maps


def _assemble(results):
    attended = np.empty((B, R, D), np.float32)
    aw = np.empty((B, R, R), np.float32)
    for c, r in enumerate(results):
        s0 = SLOC * c
        att = r["attout"].reshape(B, SLOC, D)
        attended[:, s0:s0 + SLOC, :] = att
        a = r["awout"].reshape(4, B, NQ, R).transpose(1, 2, 0, 3).reshape(B, SLOC, R)
        aw[:, s0:s0 + SLOC, :] = a
    return attended, aw


def _execute(inputs, **kwargs):
    nc = _get_nc()
    args = {k: np.asarray(inputs[k], np.float32) for k in
            ("q_embed", "kv_embed", "Wk", "bk", "Wv", "bv", "Wq", "bq",
             "Wr", "br")}
    in_maps = _make_in_maps(**args)
    res = run_bass_kernel_spmd(nc, in_maps, core_ids=list(range(NCORES)),
                               **kwargs)
    return _assemble(res.results), res


def kernel(**inputs):
    (attended, aw), _ = _execute(inputs)
    return attended, aw


# revision 8
# speedup vs baseline: 2.2868x; 2.2868x over previous
"""Trainium2 Bass kernel for nn_CustomAttention (B=16, R=128, D=128, BD=64).

Sharding: Wq (R,R,D,D) is split along the target-region axis s across the
8 cores (16 s-values per core).  Each core computes its slice of
Q/scores/attended; kv_embed and the shared K/V projections are replicated.

Per-core device layout: the 16 local s-values are processed as 4 "quads";
a quad's 4 members occupy 16-row blocks of PSUM at partition bases
0/32/64/96 (PE column-group alignment -- only 32-aligned output bases are
legal), with batch b in the 16 rows of each block.  The gap rows carry
benign garbage that is never read downstream.

The Wq stream (the memory-bound term) is shipped as a SINGLE bf16 plane
(64 MiB/core -- half the fp32 bytes; fp32 PSUM accumulation keeps the
final error well inside the 2e-2 gate), laid out host-side as
[tcx][i][s][(t,d)] so each t-chunk is one contiguous 4 MiB block, fetched
as a 2 MiB HWDGE (sync) DMA + a 2 MiB SWDGE (gpsimd) DMA in parallel --
the scalar/ACT queue carries no steady-state DMAs so score epilogues
never delay the stream.

Per (q,g) PSUM group: one M=128 bias matmul (sel2 selectors, K=16,
issued first as PSUM init -- needs only bq) then 4 col-tiled M=16 Wq
matmuls accumulate on top.  K is produced upfront into 16 per-t-chunk
tiles in the score layout (rows 32j+b) via 4-position col-tiled matmuls
against strided kv slices; bk folds into the PSUM->SBUF epilogue add.
Scores: ACT copies each PSUM Q-block to SBUF bf16, DVE does the bf16 mul
(2x mode) + one 1024-wide segmented reduce per (tcx,q).  V path runs
early to fill the stream lead-in.
"""

import numpy as np
import ml_dtypes

try:
    import concourse  # noqa: F401
except ImportError:  # pragma: no cover
    import sys

    sys.path.insert(0, "/opt/trn_rl_repo")

from contextlib import ExitStack

import concourse.mybir as mybir
import concourse.tile as tile
from concourse import bacc
from concourse.bass_utils import run_bass_kernel_spmd
from concourse.masks import make_identity

F32 = mybir.dt.float32
F32R = mybir.dt.float32r
BF16 = mybir.dt.bfloat16
AF = mybir.ActivationFunctionType
ALU = mybir.AluOpType
AXL = mybir.AxisListType
NPBF16 = ml_dtypes.bfloat16

B, R, D, BD = 16, 128, 128, 64
NCORES = 8
SLOC = R // NCORES          # 16 s-values per core
NQ = SLOC // 4              # 4 quads of 4 members
TCH = 8                     # t-values per DMA chunk
NTC = R // TCH              # 16 chunks
GP = TCH // 4               # psum groups (of 4 t) per chunk
CHF = TCH * D               # free elements per chunk per s (1024)
SH = SLOC // 2              # s-values per half-DMA

_CACHE = {}


def _build():
    nc = bacc.Bacc("TRN2", target_bir_lowering=False, debug=False,
                   enable_asserts=True, num_devices=NCORES)

    def dram_in(name, shape, dt):
        return nc.dram_tensor(name, shape, dt, kind="ExternalInput").ap()

    wqh_ap = dram_in("wqh", [NTC, D, SLOC * CHF], BF16)  # [tc][i][(s,t,d)]
    qt_ap = dram_in("qt", [D, SLOC * B], BF16)          # [i][(s,b)]
    bq_ap = dram_in("bqd", [SLOC, R * D], BF16)         # [s][(t,d)]
    kvtb_ap = dram_in("kvtb", [D, B * R], BF16)         # [i][(b,t)]
    wkb_ap = dram_in("wkb", [D, D], BF16)
    bk5_ap = dram_in("bk5", [128, 512], F32)            # bk tiled 4x along free
    wvb_ap = dram_in("wvb", [D, BD], BF16)
    wr_ap = dram_in("wr", [BD, D], F32R)
    sel2_ap = dram_in("sel2", [SLOC, NQ * 128], BF16)   # bias row selectors (K=16)
    bv_ap = dram_in("bvr", [R, BD], F32)
    br_ap = dram_in("brr", [R, D], F32)

    awout_ap = nc.dram_tensor("awout", [4, B, NQ * R], F32,
                              kind="ExternalOutput").ap()
    attout_ap = nc.dram_tensor("attout", [2, 128, D], F32,
                               kind="ExternalOutput").ap()

    with tile.TileContext(nc) as tc:
        with ExitStack() as ctx:
            per = ctx.enter_context(tc.tile_pool(name="persist", bufs=1))
            pre = ctx.enter_context(tc.tile_pool(name="prelude", bufs=2))
            wqp = ctx.enter_context(tc.tile_pool(name="wqpool", bufs=3))
            qsp = ctx.enter_context(tc.tile_pool(name="qspool", bufs=3))
            tmpp = ctx.enter_context(tc.tile_pool(name="tmppool", bufs=2))
            qpsp = ctx.enter_context(tc.tile_pool(name="qps", bufs=4, space="PSUM"))
            aux = ctx.enter_context(tc.tile_pool(name="aux", bufs=2, space="PSUM"))
            kaux = ctx.enter_context(tc.tile_pool(name="kaux", bufs=1, space="PSUM"))
            aux1 = ctx.enter_context(tc.tile_pool(name="aux1", bufs=1, space="PSUM"))

            # ---- small inputs: sync ring ahead of the wq stream ----
            bq_all = per.tile([SLOC, R * D], BF16)
            qt = per.tile([D, SLOC * B], BF16)
            sel2 = per.tile([SLOC, NQ * 128], BF16)
            wkb = per.tile([D, D], BF16)
            bk5 = per.tile([128, 512], F32)
            kvtb = per.tile([D, B * R], BF16)
            for t, ap in ((bq_all, bq_ap), (qt, qt_ap), (sel2, sel2_ap),
                          (wkb, wkb_ap), (bk5, bk5_ap), (kvtb, kvtb_ap)):
                nc.sync.dma_start(t[:], ap[:])
            # V/postlude-only inputs on the scalar ring (idle at start)
            wvb = per.tile([D, BD], BF16)
            wr = per.tile([BD, D], F32R)
            bv_rep = per.tile([R, BD], F32)
            br_rep = per.tile([R, D], F32)
            ident = per.tile([128, 128], F32)
            for t, ap in ((wvb, wvb_ap), (wr, wr_ap), (bv_rep, bv_ap),
                          (br_rep, br_ap)):
                nc.scalar.dma_start(t[:], ap[:])
            make_identity(nc, ident[:])

            kvt3 = kvtb[:].rearrange("i (b t) -> i t b", b=B)  # strided view

            # ---- K in score layout: krep[32j+b, (t,d)] = K[b,t,d] (bf16) ----
            kreps = [per.tile([128, CHF], BF16, name=f"krep{i}",
                              tag=f"krep{i}") for i in range(NTC)]
            for tcx in range(NTC):
                krep = kreps[tcx]
                for g in range(GP):
                    kps = kaux.tile([128, 512], F32, tag="kps")
                    for v in range(4):
                        t = tcx * TCH + g * 4 + v
                        for jj in range(4):
                            nc.tensor.matmul(
                                kps[32 * jj:32 * jj + B, 128 * v:128 * (v + 1)],
                                kvt3[:, t, :], wkb[:],
                                start=True, stop=True,
                                tile_position=(0, 32 * jj))
                    nc.vector.tensor_add(krep[:, g * 512:(g + 1) * 512],
                                         kps[:], bk5[:])

            # ---- V path early: V = kv@Wv + bv, row-normalized ----
            vn = per.tile([R, B * BD], F32R)            # V_norm[b] as [t, (b,dd)]
            for b in range(B):
                vb_ps = aux.tile([128, BD], F32, tag="auxA")
                nc.tensor.matmul(vb_ps[:], kvtb[:, b * R:(b + 1) * R], wvb[:],
                                 start=True, stop=True)
                vsb = pre.tile([R, BD], F32, tag="vsb")
                nc.vector.tensor_add(vsb[:], vb_ps[:], bv_rep[:])
                vsq = pre.tile([R, BD], F32, tag="vsq")
                ss = pre.tile([R, 1], F32, tag="ss")
                nc.scalar.activation(vsq[:], vsb[:], AF.Square, accum_out=ss[:])
                nrm = pre.tile([R, 1], F32, tag="nrm")
                nc.scalar.activation(nrm[:], ss[:], AF.Sqrt)
                nc.vector.tensor_scalar_max(nrm[:], nrm[:], 1e-12)
                vri = pre.tile([R, 1], F32, tag="vri")
                nc.vector.reciprocal(vri[:], nrm[:])
                nc.vector.tensor_scalar_mul(vn[:, b * BD:(b + 1) * BD], vsb[:], vri[:])

            aw_raw = per.tile([128, NQ * R], F32)       # row 32j+b, col q*128+t

            # ---- main loop: stream Wq, Q-projection + scores ----
            for tcx in range(NTC):
                wq = wqp.tile([D, SLOC * CHF], BF16, tag="wqt")
                half = SH * CHF
                nc.sync.dma_start(wq[:, 0:half], wqh_ap[tcx, :, 0:half])
                nc.gpsimd.dma_start(wq[:, half:2 * half],
                                    wqh_ap[tcx, :, half:2 * half])
                krep = kreps[tcx]
                for q in range(NQ):
                    tmp = tmpp.tile([128, CHF], BF16, tag="tmp")
                    for g in range(GP):
                        qps = qpsp.tile([128, 512], F32, tag="qps")
                        # bias first: PSUM init from bq alone (no wq dep)
                        nc.tensor.matmul(qps[:], sel2[:, q * 128:(q + 1) * 128],
                                         bq_all[:, tcx * CHF + g * 512:
                                                tcx * CHF + (g + 1) * 512],
                                         start=True, stop=False,
                                         skip_group_check=True)
                        for j in range(4):
                            s = 4 * q + j
                            csl = slice(s * CHF + g * 512, s * CHF + (g + 1) * 512)
                            nc.tensor.matmul(qps[32 * j:32 * j + B, :],
                                             qt[:, s * B:(s + 1) * B], wq[:, csl],
                                             start=False, stop=True,
                                             skip_group_check=True,
                                             tile_position=(0, 32 * j))
                        qs = qsp.tile([128, 512], BF16, tag="qs")
                        nc.scalar.activation(qs[:], qps[:], AF.Copy)
                        nc.vector.tensor_mul(tmp[:, g * 512:(g + 1) * 512],
                                             qs[:], krep[:, g * 512:(g + 1) * 512])
                    col = q * R + tcx * TCH
                    nc.vector.tensor_reduce(
                        aw_raw[:, col:col + TCH],
                        tmp[:].rearrange("p (t d) -> p t d", t=TCH),
                        axis=AXL.X, op=ALU.add)

            # ---- postlude: relu, normalize, attended, output proj ----
            aw_norm = per.tile([128, NQ * R], F32)
            ssum = per.tile([128, NQ], F32)
            rden = per.tile([128, NQ], F32)
            rinv = per.tile([128, NQ], F32)
            awt = per.tile([128, NQ * R], F32R)

            for q in range(NQ):
                nc.scalar.activation(aw_norm[:, q * R:(q + 1) * R],
                                     aw_raw[:, q * R:(q + 1) * R], AF.Relu,
                                     accum_out=ssum[:, q:q + 1])
            nc.vector.tensor_scalar_add(rden[:], ssum[:], 1e-8)
            nc.vector.reciprocal(rinv[:], rden[:])
            for q in range(NQ):
                nc.scalar.activation(aw_norm[:, q * R:(q + 1) * R],
                                     aw_norm[:, q * R:(q + 1) * R], AF.Copy,
                                     scale=rinv[:, q:q + 1])
            for j in range(4):
                nc.sync.dma_start(awout_ap[j], aw_norm[32 * j:32 * j + B, :])
            for q in range(NQ):
                trp = aux.tile([128, 128], F32, tag="auxA")
                nc.tensor.transpose(trp[:], aw_norm[:, q * R:(q + 1) * R], ident[:])
                nc.vector.tensor_copy(awt[:, q * R:(q + 1) * R], trp[:])

            attps = aux1.tile([BD, B * SLOC], F32, tag="auxC")
            awt4 = awt[:].rearrange("p (q j c) -> p q j c", q=NQ, j=4, c=32)
            for b in range(B):
                nc.tensor.matmul(attps[:, b * SLOC:(b + 1) * SLOC],
                                 vn[:, b * BD:(b + 1) * BD], awt4[:, :, :, b],
                                 start=True, stop=True)
            atts = per.tile([BD, B * SLOC], F32R)
            nc.vector.tensor_copy(atts[:], attps[:])

            for h in range(2):
                finps = aux.tile([128, D], F32, tag="auxA")
                nc.tensor.matmul(finps[:], atts[:, h * 128:(h + 1) * 128], wr[:],
                                 start=True, stop=True)
                fin = per.tile([128, D], F32, tag=f"fin{h}")
                nc.vector.tensor_add(fin[:], finps[:], br_rep[:])
                nc.sync.dma_start(attout_ap[h], fin[:])

    nc.compile()
    return nc


def _get_nc():
    if "nc" not in _CACHE:
        _CACHE["nc"] = _build()
    return _CACHE["nc"]


def _make_in_maps(q_embed, kv_embed, Wk, bk, Wv, bv, Wq, bq, Wr, br):
    sel2 = np.zeros((SLOC, NQ * 128), NPBF16)
    for q in range(NQ):
        for j in range(4):
            s = 4 * q + j
            sel2[s, q * 128 + 32 * j:q * 128 + 32 * j + B] = 1.0
    shared = {
        "kvtb": np.ascontiguousarray(
            kv_embed.transpose(2, 0, 1).reshape(D, B * R)).astype(NPBF16),
        "wkb": np.ascontiguousarray(Wk).astype(NPBF16),
        "bk5": np.ascontiguousarray(
            np.broadcast_to(np.tile(bk, 4), (128, 512))).astype(np.float32),
        "wvb": np.ascontiguousarray(Wv).astype(NPBF16),
        "wr": np.ascontiguousarray(Wr),
        "sel2": sel2,
        "bvr": np.ascontiguousarray(np.broadcast_to(bv, (R, BD))),
        "brr": np.ascontiguousarray(np.broadcast_to(br, (R, D))),
    }
    in_maps = []
    for c in range(NCORES):
        s0 = SLOC * c
        m = dict(shared)
        # [s,t,i,d] -> [tcx, i, s, tch, d] so each tcx slice is one
        # contiguous 4 MiB bf16 block with i on partitions
        wq_c = (Wq[s0:s0 + SLOC]
                .reshape(SLOC, NTC, TCH, D, D)
                .transpose(1, 3, 0, 2, 4)
                .reshape(NTC, D, SLOC * CHF))
        m["wqh"] = np.ascontiguousarray(wq_c).astype(NPBF16)
        m["qt"] = np.ascontiguousarray(
            q_embed[:, s0:s0 + SLOC, :].transpose(2, 1, 0)).reshape(
                D, SLOC * B).astype(NPBF16)
        m["bqd"] = np.ascontiguousarray(
            bq[s0:s0 + SLOC]).reshape(SLOC, R * D).astype(NPBF16)
        in_maps.append(m)
    return in_maps


def _assemble(results):
    attended = np.empty((B, R, D), np.float32)
    aw = np.empty((B, R, R), np.float32)
    for c, r in enumerate(results):
        s0 = SLOC * c
        att = r["attout"].reshape(B, SLOC, D)
        attended[:, s0:s0 + SLOC, :] = att
        a = r["awout"].reshape(4, B, NQ, R).transpose(1, 2, 0, 3).reshape(B, SLOC, R)
        aw[:, s0:s0 + SLOC, :] = a
    return attended, aw


def _execute(inputs, **kwargs):
    nc = _get_nc()
    args = {k: np.asarray(inputs[k], np.float32) for k in
            ("q_embed", "kv_embed", "Wk", "bk", "Wv", "bv", "Wq", "bq",
             "Wr", "br")}
    in_maps = _make_in_maps(**args)
    res = run_bass_kernel_spmd(nc, in_maps, core_ids=list(range(NCORES)),
                               **kwargs)
    return _assemble(res.results), res


def kernel(**inputs):
    (attended, aw), _ = _execute(inputs)
    return attended, aw
